# revision 3
# baseline (speedup 1.0000x reference)
"""Causal self-attention block (nn_CrossAttention) on 8 TRN2 NeuronCores.

Sharding: data-parallel over batch (B=2 -> 2 groups of 4 cores), tensor-parallel
over heads within a group (16 heads -> 4 heads/core, splitting Wq/Wk/Wv rows and
Wp columns). Each core computes a full [N, DIM] partial of the output projection
for its 4 heads; the host sums the 4 partials per batch and adds the bias.

Device-side layout ("transposed world", everything feature-major):
  xT   [C=1024, N=2048]     QT/KT = W @ xT -> [d, n] with d on partitions
  V    [l, d] computed DIRECTLY per 128-key-block: V_blk = xT_blk.T @ WvT
       (no PE transposes), then packed per head with a 64-wide ones block
       ([V_h|ones] even heads, [ones|V_h] odd) for fused row-sums.
  S^T  = K_j @ Q^T per (chunk, j) -> [l, n] in PSUM; the two heads of a pair
         run CONCURRENTLY in PE row groups h0/h64 (64-deep contractions).
  P^T  = exp(SCALE*S^T) -> SBUF bf16 (both heads in one ACTIVATE),
         causal-masked by a 0/1 multiply on the diagonal block
  O''  = [V_j|ones].T @ P^T accumulated per 512-query chunk: O rows + row-sums
  1/s  = reciprocal_approx_fast on the DVE (scalar engine stays exp-only)
  out  = (O/s).T-pair @ WpT -> [n, e] partial, bf16 to DRAM (host sums in f32)

Schedule: ONE long S-stream over chunks in pair-interleaved order
(0,0),(1,0),(0,1),(1,1),... paced by the scalar engine's exp throughput.
All other PE work -- O-runs of the previous chunk, Q/K/V projections, and
the output projection -- is drip-fed between S matmuls from per-chunk fill
lists sized to the exp-vs-PE deficit, so the PE never idles.  Input DMAs are
split n-major so the first projection starts after ~1.5 MB lands.
No max-subtraction is needed in the softmax (logits*scale max ~8).
"""

import numpy as np
import ml_dtypes

B = 2
N = 2048
DIM = 1024
H = 16
D = 64
SCALE = D ** -0.5
NCORES = 8
HPC = 4          # heads per core
FPC = HPC * D    # feature rows per core (256)

NB = N // 128    # 16 key blocks
KC = DIM // 128  # 8 contraction chunks
NCH = N // 512   # 4 query chunks per pair

_BF = ml_dtypes.bfloat16

_built = None


def _build():
    import concourse.bass as bass
    import concourse.mybir as mybir
    import concourse.tile as tile
    from concourse import bacc
    from contextlib import ExitStack

    bf16 = mybir.dt.bfloat16
    f32 = mybir.dt.float32
    Exp = mybir.ActivationFunctionType.Exp

    nc = bacc.Bacc()
    xT_d = nc.dram_tensor("xT", [DIM, N], bf16, kind="ExternalInput")
    wqT_d = nc.dram_tensor("wqT", [DIM, FPC], bf16, kind="ExternalInput")
    wkT_d = nc.dram_tensor("wkT", [DIM, FPC], bf16, kind="ExternalInput")
    wvT_d = nc.dram_tensor("wvT", [DIM, FPC], bf16, kind="ExternalInput")
    wpT_d = nc.dram_tensor("wpT", [FPC, DIM], bf16, kind="ExternalInput")
    mask_d = nc.dram_tensor("mask01", [128, 128], bf16, kind="ExternalInput")
    out_d = nc.dram_tensor("out", [N, DIM], bf16, kind="ExternalOutput")

    with tile.TileContext(nc) as tc, ExitStack() as ctx:
        sing = ctx.enter_context(tc.tile_pool(name="sing", bufs=1))
        pspool = ctx.enter_context(tc.tile_pool(name="pspool", bufs=3, space="PSUM"))
        o2pool = ctx.enter_context(tc.tile_pool(name="o2pool", bufs=1, space="PSUM"))
        ptpool = ctx.enter_context(tc.tile_pool(name="ptpool", bufs=2))
        rcpool = ctx.enter_context(tc.tile_pool(name="rcpool", bufs=2))
        outpool = ctx.enter_context(tc.tile_pool(name="outpool", bufs=3))

        xTs = sing.tile([128, KC, N], bf16)
        wqTs = sing.tile([128, KC, FPC], bf16)
        wkTs = sing.tile([128, KC, FPC], bf16)
        wvTs = sing.tile([128, KC, FPC], bf16)
        wpTs = sing.tile([128, 2, DIM], bf16)
        # q/k: [d(128: even head 0:64 / odd 64:128), pair t, 512-col group, 512]
        qTs = sing.tile([128, 2, 4, 512], bf16)
        kTs = sing.tile([128, 2, 4, 512], bf16)
        # v2: per (key block j, head h) a 128-col weight slot:
        # even h -> [V_h | ones], odd h -> [ones | V_h]
        v2 = sing.tile([128, NB, HPC, 128], bf16)
        onorm = sing.tile([128, 2, N], bf16)
        maskS = sing.tile([128, 128], bf16)

        # ---- input DMAs, first-needed first; x split n-major so the first
        # projection waits for ~1.5 MB, not the whole 6 MB ----
        nc.sync.dma_start(out=maskS, in_=mask_d[:, :])
        nc.sync.dma_start(
            out=wqTs[:, :, 0:128],
            in_=wqT_d[:, 0:128].rearrange("(a p) d -> p a d", p=128))
        nc.sync.dma_start(
            out=wkTs[:, :, 0:128],
            in_=wkT_d[:, 0:128].rearrange("(a p) d -> p a d", p=128))
        for g in range(4):
            n0 = 512 * g
            nc.sync.dma_start(
                out=xTs[:, :, n0:n0 + 512],
                in_=xT_d[:, n0:n0 + 512].rearrange("(a p) n -> p a n", p=128))
            if g == 0:
                nc.sync.dma_start(
                    out=wvTs, in_=wvT_d[:].rearrange("(a p) d -> p a d", p=128))
        nc.sync.dma_start(
            out=wqTs[:, :, 128:256],
            in_=wqT_d[:, 128:256].rearrange("(a p) d -> p a d", p=128))
        nc.sync.dma_start(
            out=wkTs[:, :, 128:256],
            in_=wkT_d[:, 128:256].rearrange("(a p) d -> p a d", p=128))
        nc.sync.dma_start(out=wpTs, in_=wpT_d[:].rearrange("(a p) d -> p a d", p=128))

        for h in range(HPC):
            ones_cols = slice(64, 128) if h % 2 == 0 else slice(0, 64)
            nc.vector.memset(v2[:, :, h, ones_cols], 1.0)

        # ---- fill units: independent PE work drip-fed into S-run stalls ----
        def qk_unit(wt, dst, t, g):
            """Project one 512-col group of Q or K for pair t (8 matmuls)."""
            ps = pspool.tile([128, 512], f32, tag="ps", name="qk_ps")
            n0 = 512 * g
            for k in range(KC):
                nc.tensor.matmul(
                    ps[:, :],
                    lhsT=wt[:, k, 128 * t:128 * (t + 1)],
                    rhs=xTs[:, k, n0:n0 + 512],
                    start=(k == 0), stop=(k == KC - 1),
                )
            nc.vector.tensor_copy(out=dst[:, t, g, :], in_=ps[:, :])

        def v_block(j):
            # V_blk[l, d of all 4 heads] = sum_k xT[k-chunk, blk].T @ WvT[k-chunk]
            vps = pspool.tile([128, 256], f32, tag="ps", name="vps")
            for k in range(KC):
                nc.tensor.matmul(
                    vps[:, :],
                    lhsT=xTs[:, k, 128 * j:128 * (j + 1)],
                    rhs=wvTs[:, k, :],
                    start=(k == 0), stop=(k == KC - 1),
                )
            # scatter each pair's two heads into their [V|ones]/[ones|V] slots
            part_d = list(v2[:, :, :, :].ap)[0]
            part_s = list(vps[:, :].ap)[0]
            for t in range(2):
                dst = bass.AP(
                    tensor=v2.tensor,
                    offset=v2.offset + j * HPC * 128 + 256 * t,
                    ap=[[part_d[0], part_d[1]], [192, 2], [1, 64]],
                )
                src = bass.AP(
                    tensor=vps.tensor,
                    offset=vps.offset + 128 * t,
                    ap=[[part_s[0], 128], [64, 2], [1, 64]],
                )
                nc.vector.tensor_copy(out=dst, in_=src)

        def out_proj_nb(nb):
            po = pspool.tile([128, 2, 512], f32, tag="ps", name="po")
            for half in range(2):
                for p in range(2):
                    nc.tensor.matmul(
                        po[:, half, :],
                        lhsT=onorm[:, p, 128 * nb:128 * (nb + 1)],
                        rhs=wpTs[:, p, 512 * half:512 * half + 512],
                        start=(p == 0), stop=(p == 1),
                    )
            ostage = outpool.tile([128, 2, 512], bf16, tag="ostage", name="ostage")
            nc.vector.tensor_copy(out=ostage, in_=po)
            nc.sync.dma_start(
                out=out_d[128 * nb:128 * (nb + 1), :],
                in_=ostage.rearrange("p a b -> p (a b)"),
            )

        # ---- attention pieces ----
        state = {"pt": {}, "o2": {}}

        def s_block(t, c, j):
            """One key block j of chunk (t, c): S pair matmuls (row-grouped,
            concurrent) + exp + diagonal mask.  Returns its PE deficit."""
            c0 = 512 * c
            o = max(0, 128 * j - c0)
            w = 512 - o
            pt = state["pt"][(t, c)]
            st = pspool.tile([128, 2, 512], f32, tag="ps", name="st")
            for par in range(2):
                nc.tensor.matmul(
                    st[:, par, o:],
                    lhsT=kTs[64 * par:64 * par + 64, t, j // 4,
                             128 * (j % 4):128 * (j % 4) + 128],
                    rhs=qTs[64 * par:64 * par + 64, t, c, o:],
                    start=True, stop=True,
                )
            nc.scalar.activation(
                out=pt[:, j, :, o:], in_=st[:, :, o:],
                func=Exp, scale=SCALE,
            )
            if 128 * j >= c0:  # diagonal block: zero strictly-lower (l>n)
                for par in range(2):
                    nc.vector.tensor_mul(
                        pt[:, j, par, o:o + 128],
                        pt[:, j, par, o:o + 128],
                        maskS,
                    )
            # exp time minus the (row-group concurrent) S pair
            return max(0, int((2 * w + 344) / 1.2 + 50 - (w / 2.4 + 190)))

        def o_block(t, c, j):
            """One key block of the O-run for chunk (t, c): 2 matmuls
            accumulating into o2."""
            c0 = 512 * c
            jc = 4 * c + 4
            o = max(0, 128 * j - c0)
            pt = state["pt"][(t, c)]
            if j == 0:
                state["o2"][(t, c)] = o2pool.tile(
                    [128, 2, 512], f32, tag="o2", name="o2")
            o2 = state["o2"][(t, c)]
            for par in range(2):
                nc.tensor.matmul(
                    o2[:, par, o:],
                    lhsT=v2[:, j, 2 * t + par, :],
                    rhs=pt[:, j, par, o:],
                    start=(j == 0), stop=(j == jc - 1),
                )
            if j == jc - 1:
                norm(t, c)

        def norm(t, c):
            """1/s via the native DVE reciprocal (full 128-partition width),
            sums staged to SBUF + partition-shuffled by DMA first.  No
            scalar-engine involvement."""
            c0 = 512 * c
            o2 = state["o2"].pop((t, c))
            rc = rcpool.tile([128, 3, 512], f32, tag="rc", name="rc")
            # stage row sums (DVE: PSUM -> SBUF; DMA cannot read PSUM)
            nc.vector.tensor_copy(out=rc[64:128, 0, :], in_=o2[64:128, 0, :])
            nc.vector.tensor_copy(out=rc[0:64, 0, :], in_=o2[0:64, 1, :])
            # move sums onto O's partitions (DMA shuffles partitions)
            nc.sync.dma_start(out=rc[0:64, 1, :], in_=rc[64:128, 0, :])
            nc.sync.dma_start(out=rc[64:128, 1, :], in_=rc[0:64, 0, :])
            nc.vector.reciprocal(out=rc[:, 2, :], in_=rc[:, 1, :])
            nc.vector.tensor_mul(
                out=onorm[0:64, t, c0:c0 + 512], in0=o2[0:64, 0, :],
                in1=rc[0:64, 2, :],
            )
            nc.vector.tensor_mul(
                out=onorm[64:128, t, c0:c0 + 512], in0=o2[64:128, 1, :],
                in1=rc[64:128, 2, :],
            )

        # ================= static fill schedule =================
        # Chunks run pair-interleaved: (0,0),(1,0),(0,1),(1,1),...  FILLS[i]
        # is the PE work drip-fed into chunk i's S-run (debt-paced, leftovers
        # emitted at S-run end).  O(chunk i) units land in FILLS[i+1] after
        # that chunk's qk units (so norm(i-1) has drained before o2 reuse).
        QK = 1780   # 8x512 matmul unit, ns
        VB = 980    # v_block
        OP = 970    # out_proj_nb

        def qk_u(wt, dst, t, g):
            return (QK, lambda: qk_unit(wt, dst, t, g))

        def v_u(j):
            return (VB, lambda jj=j: v_block(jj))

        def op_u(nb):
            return (OP, lambda b=nb: out_proj_nb(b))

        def o_us(t, c):
            jc = 4 * c + 4
            return [(int(2 * (512 - max(0, 128 * j - 512 * c)) / 2.4) + 200,
                     (lambda tt=t, cc=c, jj=j: o_block(tt, cc, jj)))
                    for j in range(jc)]

        seq = [(0, 0), (1, 0), (0, 1), (1, 1), (0, 2), (1, 2), (0, 3), (1, 3)]
        FILLS = {
            (0, 0): [qk_u(wqTs, qTs, 1, 0), qk_u(wkTs, kTs, 1, 0),
                     v_u(0), v_u(1), v_u(2), v_u(3)],
            (1, 0): [qk_u(wqTs, qTs, 0, 1), qk_u(wkTs, kTs, 0, 1)]
                    + o_us(0, 0),
            (0, 1): [qk_u(wqTs, qTs, 1, 1), qk_u(wkTs, kTs, 1, 1)]
                    + o_us(1, 0) + [v_u(4), v_u(5), v_u(6), v_u(7)],
            (1, 1): [qk_u(wqTs, qTs, 0, 2), qk_u(wkTs, kTs, 0, 2)]
                    + o_us(0, 1) + [op_u(0), op_u(1)],
            (0, 2): [qk_u(wqTs, qTs, 1, 2), qk_u(wkTs, kTs, 1, 2)]
                    + o_us(1, 1) + [v_u(8), v_u(9), v_u(10), v_u(11),
                                    op_u(2), op_u(3)],
            (1, 2): [qk_u(wqTs, qTs, 0, 3), qk_u(wkTs, kTs, 0, 3)]
                    + o_us(0, 2) + [op_u(4), op_u(5), op_u(6), op_u(7)],
            (0, 3): [qk_u(wqTs, qTs, 1, 3), qk_u(wkTs, kTs, 1, 3)]
                    + o_us(1, 2) + [v_u(12), v_u(13), v_u(14), v_u(15),
                                    op_u(8), op_u(9)],
            (1, 3): o_us(0, 3) + [op_u(10), op_u(11)],
        }
        DRAIN = o_us(1, 3) + [op_u(12), op_u(13), op_u(14), op_u(15)]

        # upfront: minimal projections for chunk (0,0)
        qk_unit(wqTs, qTs, 0, 0)
        qk_unit(wkTs, kTs, 0, 0)

        debt = 0
        for t, c in seq:
            fills = list(FILLS[(t, c)])
            state["pt"][(t, c)] = ptpool.tile(
                [128, NB, 2, 512], bf16, tag="pt", name="pt")
            jc = 4 * c + 4
            for j in range(jc):
                debt += s_block(t, c, j)
                while fills and debt >= fills[0][0]:
                    cost, f = fills.pop(0)
                    debt -= cost
                    f()
                debt = min(debt, 4000)
            # leftovers must land before the next chunk's S-run
            for cost, f in fills:
                f()
            # pt of the chunk before last is now fully consumed
            prv = seq[seq.index((t, c)) - 2]
            if seq.index((t, c)) >= 2:
                state["pt"].pop(prv, None)
        for cost, f in DRAIN:
            f()

    nc.finalize()
    return nc


def _get_nc():
    global _built
    if _built is None:
        _built = _build()
    return _built


def make_in_maps(x, Wq, Wk, Wv, Wp):
    # 0 where key>query (strictly-lower in [l, n] coords), else 1
    mask = np.where(
        np.arange(128)[:, None] > np.arange(128)[None, :], 0.0, 1.0
    ).astype(_BF)
    in_maps = []
    for c in range(NCORES):
        b, g = c // HPC, c % HPC
        rows = slice(FPC * g, FPC * (g + 1))
        in_maps.append({
            "xT": np.ascontiguousarray(x[b].T).astype(_BF),
            "wqT": np.ascontiguousarray(Wq[rows, :].T).astype(_BF),
            "wkT": np.ascontiguousarray(Wk[rows, :].T).astype(_BF),
            "wvT": np.ascontiguousarray(Wv[rows, :].T).astype(_BF),
            "wpT": np.ascontiguousarray(Wp[:, rows].T).astype(_BF),
            "mask01": mask,
        })
    return in_maps


def run_sharded(x, Wq, Wk, Wv, Wp, bp, trace=False, **spmd_kwargs):
    from concourse.bass_utils import run_bass_kernel_spmd

    nc = _get_nc()
    in_maps = make_in_maps(x, Wq, Wk, Wv, Wp)
    res = run_bass_kernel_spmd(
        nc, in_maps, core_ids=list(range(NCORES)), trace=trace, **spmd_kwargs
    )
    parts = [r["out"] for r in res.results]
    out = np.zeros((B, N, DIM), np.float32)
    for b in range(B):
        acc = np.zeros((N, DIM), np.float32)
        for g in range(HPC):
            acc += np.asarray(parts[b * HPC + g], dtype=np.float32)
        out[b] = acc + bp.astype(np.float32)[None, :]
    return out, res


def kernel(x, y, Wq, Wk, Wv, Wp, bp):
    x = np.asarray(x, np.float32)
    out, _ = run_sharded(
        x,
        np.asarray(Wq, np.float32), np.asarray(Wk, np.float32),
        np.asarray(Wv, np.float32), np.asarray(Wp, np.float32),
        np.asarray(bp, np.float32),
    )
    return out


# revision 6
# speedup vs baseline: 1.3358x; 1.3358x over previous
"""Causal self-attention block (nn_CrossAttention) on 8 TRN2 NeuronCores.

Sharding: data-parallel over batch (B=2 -> 2 groups of 4 cores), tensor-parallel
over heads within a group (16 heads -> 4 heads/core, splitting Wq/Wk/Wv rows and
Wp columns). Each core computes a full [N, DIM] partial of the output projection
for its 4 heads; the host sums the 4 partials per batch and adds the bias.

Device-side layout ("transposed world", everything feature-major):
  xT   [C=1024, N=2048]     QT/KT = W @ xT -> [d, n] with d on partitions
  V    [l, d] computed DIRECTLY per 128-key-block: V_blk = xT_blk.T @ WvT
       (no PE transposes), then packed per head with a 64-wide ones block
       ([V_h|ones] even heads, [ones|V_h] odd) for fused row-sums.
  S^T  = K_j @ Q^T per (chunk, j) -> [l, n] in PSUM; the two heads of a pair
         run CONCURRENTLY in PE row groups h0/h64 (64-deep contractions).
  P^T  = exp(SCALE*S^T) -> SBUF bf16 (both heads in one ACTIVATE),
         causal-masked by a 0/1 multiply on the diagonal block
  O''  = [V_j|ones].T @ P^T accumulated per 512-query chunk: O rows + row-sums
  1/s  = reciprocal_approx_fast on the DVE (scalar engine stays exp-only)
  out  = (O/s).T-pair @ WpT -> [n, e] partial, bf16 to DRAM (host sums in f32)

Schedule: ONE long S-stream over chunks in pair-interleaved order
(0,0),(1,0),(0,1),(1,1),... paced by the scalar engine's exp throughput.
All other PE work -- O-runs of the previous chunk, Q/K/V projections, and
the output projection -- is drip-fed between S matmuls from per-chunk fill
lists sized to the exp-vs-PE deficit, so the PE never idles.  Input DMAs are
split n-major so the first projection starts after ~1.5 MB lands.
No max-subtraction is needed in the softmax (logits*scale max ~8).
"""

import numpy as np
import ml_dtypes

B = 2
N = 2048
DIM = 1024
H = 16
D = 64
SCALE = D ** -0.5
NCORES = 8
HPC = 4          # heads per core
FPC = HPC * D    # feature rows per core (256)

NB = N // 128    # 16 key blocks
KC = DIM // 128  # 8 contraction chunks
NCH = N // 512   # 4 query chunks per pair

_BF = ml_dtypes.bfloat16

_built = None


def _build():
    import concourse.bass as bass
    import concourse.mybir as mybir
    import concourse.tile as tile
    from concourse import bacc
    from contextlib import ExitStack

    # The kernel's only transcendentals are Exp (softmax) and Ln (row-sum
    # reciprocal).  Left to itself the act-table pass picks "exp_and_others"
    # for Exp and "natural_log" for Ln, reloading tables (~1.3us, serializing
    # the scalar engine) on every chunk.  Hide Exp/Ln from every set except
    # the combined one so both resolve to a single resident table.
    if not getattr(bacc, "_act_tables_pinned", False):
        orig_get = bacc.get_activation_tables

        def pinned_get(arch):
            t = {k: set(v) for k, v in orig_get(arch).items()}
            exp = mybir.ActivationFunctionType.Exp
            ln = mybir.ActivationFunctionType.Ln
            for name, fns in t.items():
                if name != "natural_log_exp_and_others":
                    fns.discard(exp)
                    fns.discard(ln)
            return t

        bacc.get_activation_tables = pinned_get
        bacc._act_tables_pinned = True

    bf16 = mybir.dt.bfloat16
    f32 = mybir.dt.float32
    Exp = mybir.ActivationFunctionType.Exp
    Ln = mybir.ActivationFunctionType.Ln

    nc = bacc.Bacc()
    xT_d = nc.dram_tensor("xT", [DIM, N], bf16, kind="ExternalInput")
    wqT_d = nc.dram_tensor("wqT", [DIM, FPC], bf16, kind="ExternalInput")
    wkT_d = nc.dram_tensor("wkT", [DIM, FPC], bf16, kind="ExternalInput")
    wvT_d = nc.dram_tensor("wvT", [DIM, FPC], bf16, kind="ExternalInput")
    wpT_d = nc.dram_tensor("wpT", [FPC, DIM], bf16, kind="ExternalInput")
    mask_d = nc.dram_tensor("mask01", [128, 128], bf16, kind="ExternalInput")
    out_d = nc.dram_tensor("out", [N, DIM], bf16, kind="ExternalOutput")

    with tile.TileContext(nc) as tc, ExitStack() as ctx:
        sing = ctx.enter_context(tc.tile_pool(name="sing", bufs=1))
        pspool = ctx.enter_context(tc.tile_pool(name="pspool", bufs=3, space="PSUM"))
        o2pool = ctx.enter_context(tc.tile_pool(name="o2pool", bufs=1, space="PSUM"))
        ptpool = ctx.enter_context(tc.tile_pool(name="ptpool", bufs=2))
        rcpool = ctx.enter_context(tc.tile_pool(name="rcpool", bufs=2))
        outpool = ctx.enter_context(tc.tile_pool(name="outpool", bufs=3))

        xTs = sing.tile([128, KC, N], bf16)
        wqTs = sing.tile([128, KC, FPC], bf16)
        wkTs = sing.tile([128, KC, FPC], bf16)
        wvTs = sing.tile([128, KC, FPC], bf16)
        wpTs = sing.tile([128, 2, DIM], bf16)
        # q/k: [d(128: even head 0:64 / odd 64:128), pair t, 512-col group, 512]
        qTs = sing.tile([128, 2, 4, 512], bf16)
        kTs = sing.tile([128, 2, 4, 512], bf16)
        # v2: per (key block j, head h) a 128-col weight slot:
        # even h -> [V_h | ones], odd h -> [ones | V_h]
        v2 = sing.tile([128, NB, HPC, 128], bf16)
        onorm = sing.tile([128, 2, N], bf16)
        maskS = sing.tile([128, 128], bf16)

        # ---- input DMAs, first-needed first; x split n-major so the first
        # projection waits for ~1.5 MB, not the whole 6 MB ----
        nc.sync.dma_start(out=maskS, in_=mask_d[:, :])
        nc.sync.dma_start(
            out=wqTs[:, :, 0:128],
            in_=wqT_d[:, 0:128].rearrange("(a p) d -> p a d", p=128))
        nc.sync.dma_start(
            out=wkTs[:, :, 0:128],
            in_=wkT_d[:, 0:128].rearrange("(a p) d -> p a d", p=128))
        for g in range(4):
            n0 = 512 * g
            nc.sync.dma_start(
                out=xTs[:, :, n0:n0 + 512],
                in_=xT_d[:, n0:n0 + 512].rearrange("(a p) n -> p a n", p=128))
            if g == 0:
                nc.sync.dma_start(
                    out=wvTs, in_=wvT_d[:].rearrange("(a p) d -> p a d", p=128))
        nc.sync.dma_start(
            out=wqTs[:, :, 128:256],
            in_=wqT_d[:, 128:256].rearrange("(a p) d -> p a d", p=128))
        nc.sync.dma_start(
            out=wkTs[:, :, 128:256],
            in_=wkT_d[:, 128:256].rearrange("(a p) d -> p a d", p=128))
        nc.sync.dma_start(out=wpTs, in_=wpT_d[:].rearrange("(a p) d -> p a d", p=128))

        for h in range(HPC):
            ones_cols = slice(64, 128) if h % 2 == 0 else slice(0, 64)
            nc.vector.memset(v2[:, :, h, ones_cols], 1.0)

        # ---- fill units: independent PE work drip-fed into S-run stalls ----
        def qk_unit(wt, dst, t, g):
            """Project one 512-col group of Q or K for pair t (8 matmuls)."""
            ps = pspool.tile([128, 512], f32, tag="ps", name="qk_ps")
            n0 = 512 * g
            for k in range(KC):
                nc.tensor.matmul(
                    ps[:, :],
                    lhsT=wt[:, k, 128 * t:128 * (t + 1)],
                    rhs=xTs[:, k, n0:n0 + 512],
                    start=(k == 0), stop=(k == KC - 1),
                )
            nc.vector.tensor_copy(out=dst[:, t, g, :], in_=ps[:, :])

        def v_block(j):
            # V_blk[l, d of all 4 heads] = sum_k xT[k-chunk, blk].T @ WvT[k-chunk]
            vps = pspool.tile([128, 256], f32, tag="ps", name="vps")
            for k in range(KC):
                nc.tensor.matmul(
                    vps[:, :],
                    lhsT=xTs[:, k, 128 * j:128 * (j + 1)],
                    rhs=wvTs[:, k, :],
                    start=(k == 0), stop=(k == KC - 1),
                )
            # scatter each pair's two heads into their [V|ones]/[ones|V] slots
            part_d = list(v2[:, :, :, :].ap)[0]
            part_s = list(vps[:, :].ap)[0]
            for t in range(2):
                dst = bass.AP(
                    tensor=v2.tensor,
                    offset=v2.offset + j * HPC * 128 + 256 * t,
                    ap=[[part_d[0], part_d[1]], [192, 2], [1, 64]],
                )
                src = bass.AP(
                    tensor=vps.tensor,
                    offset=vps.offset + 128 * t,
                    ap=[[part_s[0], 128], [64, 2], [1, 64]],
                )
                nc.vector.tensor_copy(out=dst, in_=src)

        def out_proj_nb(nb):
            po = pspool.tile([128, 2, 512], f32, tag="ps", name="po")
            for half in range(2):
                for p in range(2):
                    nc.tensor.matmul(
                        po[:, half, :],
                        lhsT=onorm[:, p, 128 * nb:128 * (nb + 1)],
                        rhs=wpTs[:, p, 512 * half:512 * half + 512],
                        start=(p == 0), stop=(p == 1),
                    )
            ostage = outpool.tile([128, 2, 512], bf16, tag="ostage", name="ostage")
            nc.vector.tensor_copy(out=ostage, in_=po)
            nc.sync.dma_start(
                out=out_d[128 * nb:128 * (nb + 1), :],
                in_=ostage.rearrange("p a b -> p (a b)"),
            )

        # ---- attention pieces ----
        state = {"pt": {}, "o2": {}}

        def s_block(t, c, j):
            """One key block j of chunk (t, c): S pair matmuls (row-grouped,
            concurrent) + exp + diagonal mask.  Returns its PE deficit."""
            c0 = 512 * c
            o = max(0, 128 * j - c0)
            w = 512 - o
            pt = state["pt"][(t, c)]
            st = pspool.tile([128, 2, 512], f32, tag="ps", name="st")
            for par in range(2):
                nc.tensor.matmul(
                    st[:, par, o:],
                    lhsT=kTs[64 * par:64 * par + 64, t, j // 4,
                             128 * (j % 4):128 * (j % 4) + 128],
                    rhs=qTs[64 * par:64 * par + 64, t, c, o:],
                    start=True, stop=True,
                )
            nc.scalar.activation(
                out=pt[:, j, :, o:], in_=st[:, :, o:],
                func=Exp, scale=SCALE,
            )
            if 128 * j >= c0:  # diagonal block: zero strictly-lower (l>n)
                # on GPSIMD (~410ns/op): keeps the DVE free for casts/norm
                for par in range(2):
                    nc.gpsimd.tensor_mul(
                        pt[:, j, par, o:o + 128],
                        pt[:, j, par, o:o + 128],
                        maskS,
                    )
            # exp time minus the (row-group concurrent) S pair
            return max(0, int((2 * w + 344) / 1.2 + 50 - (w / 2.4 + 190)))

        def o_block(t, c, j):
            """One key block of the O-run for chunk (t, c): 2 matmuls
            accumulating into o2."""
            c0 = 512 * c
            jc = 4 * c + 4
            o = max(0, 128 * j - c0)
            pt = state["pt"][(t, c)]
            if j == 0:
                state["o2"][(t, c)] = o2pool.tile(
                    [128, 2, 512], f32, tag="o2", name="o2")
            o2 = state["o2"][(t, c)]
            for par in range(2):
                nc.tensor.matmul(
                    o2[:, par, o:],
                    lhsT=v2[:, j, 2 * t + par, :],
                    rhs=pt[:, j, par, o:],
                    start=(j == 0), stop=(j == jc - 1),
                )
            if j == jc - 1:
                norm(t, c)

        def norm(t, c):
            """1/s = exp(-ln s).  Sums staged to SBUF (DVE) and partition-
            shuffled by DMA FIRST, so Ln and Exp each run once at full
            128-partition width (1.6us of ACT per chunk vs 2.8us for the
            per-half variant; native DVE reciprocal measures ~4us/op)."""
            c0 = 512 * c
            o2 = state["o2"].pop((t, c))
            rc = rcpool.tile([128, 3, 512], f32, tag="rc", name="rc")
            # stage row sums (DVE: PSUM -> SBUF; DMA cannot read PSUM)
            nc.vector.tensor_copy(out=rc[64:128, 0, :], in_=o2[64:128, 0, :])
            nc.vector.tensor_copy(out=rc[0:64, 0, :], in_=o2[0:64, 1, :])
            # move sums onto O's partitions (DMA shuffles partitions)
            nc.sync.dma_start(out=rc[0:64, 1, :], in_=rc[64:128, 0, :])
            nc.sync.dma_start(out=rc[64:128, 1, :], in_=rc[0:64, 0, :])
            nc.scalar.activation(out=rc[:, 2, :], in_=rc[:, 1, :], func=Ln)
            nc.scalar.activation(out=rc[:, 1, :], in_=rc[:, 2, :],
                                 func=Exp, scale=-1.0)
            nc.vector.tensor_mul(
                out=onorm[0:64, t, c0:c0 + 512], in0=o2[0:64, 0, :],
                in1=rc[0:64, 1, :],
            )
            nc.vector.tensor_mul(
                out=onorm[64:128, t, c0:c0 + 512], in0=o2[64:128, 1, :],
                in1=rc[64:128, 1, :],
            )

        # ================= static fill schedule =================
        # Chunks run pair-interleaved: (0,0),(1,0),(0,1),(1,1),...  FILLS[i]
        # is the PE work drip-fed into chunk i's S-run (debt-paced, leftovers
        # emitted at S-run end).  O(chunk i) units land in FILLS[i+1] after
        # that chunk's qk units (so norm(i-1) has drained before o2 reuse).
        QK = 1780   # 8x512 matmul unit, ns
        VB = 980    # v_block
        OP = 970    # out_proj_nb

        def qk_u(wt, dst, t, g):
            return (QK, lambda: qk_unit(wt, dst, t, g))

        def v_u(j):
            return (VB, lambda jj=j: v_block(jj))

        def op_u(nb):
            return (OP, lambda b=nb: out_proj_nb(b))

        def o_us(t, c):
            jc = 4 * c + 4
            return [(int(2 * (512 - max(0, 128 * j - 512 * c)) / 2.4) + 200,
                     (lambda tt=t, cc=c, jj=j: o_block(tt, cc, jj)))
                    for j in range(jc)]

        seq = [(0, 0), (1, 0), (0, 1), (1, 1), (0, 2), (1, 2), (0, 3), (1, 3)]
        FILLS = {
            (0, 0): [qk_u(wqTs, qTs, 1, 0), qk_u(wkTs, kTs, 1, 0),
                     v_u(0), v_u(1), v_u(2), v_u(3)],
            (1, 0): [qk_u(wqTs, qTs, 0, 1), qk_u(wkTs, kTs, 0, 1)]
                    + o_us(0, 0),
            (0, 1): [qk_u(wqTs, qTs, 1, 1), qk_u(wkTs, kTs, 1, 1)]
                    + o_us(1, 0) + [v_u(4), v_u(5), v_u(6), v_u(7)],
            (1, 1): [qk_u(wqTs, qTs, 0, 2), qk_u(wkTs, kTs, 0, 2)]
                    + o_us(0, 1) + [op_u(0), op_u(1)],
            (0, 2): [qk_u(wqTs, qTs, 1, 2), qk_u(wkTs, kTs, 1, 2)]
                    + o_us(1, 1) + [v_u(8), v_u(9), v_u(10), v_u(11),
                                    op_u(2), op_u(3)],
            (1, 2): [qk_u(wqTs, qTs, 0, 3), qk_u(wkTs, kTs, 0, 3)]
                    + o_us(0, 2) + [op_u(4), op_u(5), op_u(6), op_u(7)],
            (0, 3): [qk_u(wqTs, qTs, 1, 3), qk_u(wkTs, kTs, 1, 3)]
                    + o_us(1, 2) + [v_u(12), v_u(13), v_u(14), v_u(15),
                                    op_u(8), op_u(9)],
            (1, 3): o_us(0, 3) + [op_u(10), op_u(11)],
        }
        DRAIN = o_us(1, 3) + [op_u(12), op_u(13), op_u(14), op_u(15)]

        # upfront: minimal projections for chunk (0,0)
        qk_unit(wqTs, qTs, 0, 0)
        qk_unit(wkTs, kTs, 0, 0)

        debt = 0
        for t, c in seq:
            fills = list(FILLS[(t, c)])
            state["pt"][(t, c)] = ptpool.tile(
                [128, NB, 2, 512], bf16, tag="pt", name="pt")
            jc = 4 * c + 4
            for j in range(jc):
                debt += s_block(t, c, j)
                while fills and debt >= fills[0][0]:
                    cost, f = fills.pop(0)
                    debt -= cost
                    f()
                debt = min(debt, 4000)
            # leftovers must land before the next chunk's S-run
            for cost, f in fills:
                f()
            # pt of the chunk before last is now fully consumed
            prv = seq[seq.index((t, c)) - 2]
            if seq.index((t, c)) >= 2:
                state["pt"].pop(prv, None)
        for cost, f in DRAIN:
            f()

    nc.finalize()
    return nc


def _get_nc():
    global _built
    if _built is None:
        _built = _build()
    return _built


def make_in_maps(x, Wq, Wk, Wv, Wp):
    # 0 where key>query (strictly-lower in [l, n] coords), else 1
    mask = np.where(
        np.arange(128)[:, None] > np.arange(128)[None, :], 0.0, 1.0
    ).astype(_BF)
    in_maps = []
    for c in range(NCORES):
        b, g = c // HPC, c % HPC
        rows = slice(FPC * g, FPC * (g + 1))
        in_maps.append({
            "xT": np.ascontiguousarray(x[b].T).astype(_BF),
            "wqT": np.ascontiguousarray(Wq[rows, :].T).astype(_BF),
            "wkT": np.ascontiguousarray(Wk[rows, :].T).astype(_BF),
            "wvT": np.ascontiguousarray(Wv[rows, :].T).astype(_BF),
            "wpT": np.ascontiguousarray(Wp[:, rows].T).astype(_BF),
            "mask01": mask,
        })
    return in_maps


def run_sharded(x, Wq, Wk, Wv, Wp, bp, trace=False, **spmd_kwargs):
    from concourse.bass_utils import run_bass_kernel_spmd

    nc = _get_nc()
    in_maps = make_in_maps(x, Wq, Wk, Wv, Wp)
    res = run_bass_kernel_spmd(
        nc, in_maps, core_ids=list(range(NCORES)), trace=trace, **spmd_kwargs
    )
    parts = [r["out"] for r in res.results]
    out = np.zeros((B, N, DIM), np.float32)
    for b in range(B):
        acc = np.zeros((N, DIM), np.float32)
        for g in range(HPC):
            acc += np.asarray(parts[b * HPC + g], dtype=np.float32)
        out[b] = acc + bp.astype(np.float32)[None, :]
    return out, res


def kernel(x, y, Wq, Wk, Wv, Wp, bp):
    x = np.asarray(x, np.float32)
    out, _ = run_sharded(
        x,
        np.asarray(Wq, np.float32), np.asarray(Wk, np.float32),
        np.asarray(Wv, np.float32), np.asarray(Wp, np.float32),
        np.asarray(bp, np.float32),
    )
    return out


# revision 14
# speedup vs baseline: 1.4405x; 1.0784x over previous
"""Causal self-attention block (nn_CrossAttention) on 8 TRN2 NeuronCores.

Sharding: data-parallel over batch (B=2 -> 2 groups of 4 cores), tensor-parallel
over heads within a group (16 heads -> 4 heads/core, splitting Wq/Wk/Wv rows and
Wp columns). Each core computes a full [N, DIM] partial of the output projection
for its 4 heads; the host sums the 4 partials per batch and adds the bias.

Device-side layout ("transposed world", everything feature-major):
  xT   [C=1024, N=2048]     QT/KT = W @ xT -> [d, n] with d on partitions
  V    [l, d] computed DIRECTLY per 128-key-block: V_blk = xT_blk.T @ WvT
       (no PE transposes), then packed per head with a 64-wide ones block
       ([V_h|ones] even heads, [ones|V_h] odd) for fused row-sums.
  S^T  = K_j @ Q^T per (chunk, j) -> [l, n] in PSUM; the two heads of a pair
         run CONCURRENTLY in PE row groups h0/h64 (64-deep contractions).
  P^T  = exp(SCALE*S^T) -> SBUF bf16 (both heads in one ACTIVATE),
         causal-masked by a 0/1 multiply on the diagonal block
  O''  = [V_j|ones].T @ P^T accumulated per 512-query chunk: O rows + row-sums
  1/s  = exp(-ln s), one full-width Ln+Exp pair per chunk (sums staged and
         partition-shuffled onto O's partitions first)
  out  = (O/s).T-pair @ WpT -> [n, e] partial, bf16 to DRAM (host sums in f32)

Schedule: ONE long S-stream over chunks in pair-interleaved order
(0,0),(1,0),(0,1),(1,1),... paced by the scalar engine's exp throughput.
All other PE work -- O-runs of the previous chunk, Q/K/V projections, and
the output projection -- is drip-fed between S matmuls from per-chunk fill
lists sized to the exp-vs-PE deficit, so the PE never idles.  Input DMAs are
split n-major so the first projection starts after ~1.5 MB lands.
No max-subtraction is needed in the softmax (logits*scale max ~8).
"""

import numpy as np
import ml_dtypes

B = 2
N = 2048
DIM = 1024
H = 16
D = 64
SCALE = D ** -0.5
NCORES = 8
HPC = 4          # heads per core
FPC = HPC * D    # feature rows per core (256)

NB = N // 128    # 16 key blocks
KC = DIM // 128  # 8 contraction chunks
NCH = N // 512   # 4 query chunks per pair

_BF = ml_dtypes.bfloat16

_built = None


def _build():
    import concourse.bass as bass
    import concourse.mybir as mybir
    import concourse.tile as tile
    from concourse import bacc
    from contextlib import ExitStack

    # The kernel's only transcendentals are Exp (softmax) and Ln (row-sum
    # reciprocal).  Left to itself the act-table pass picks "exp_and_others"
    # for Exp and "natural_log" for Ln, reloading tables (~1.3us, serializing
    # the scalar engine) on every chunk.  Hide Exp/Ln from every set except
    # the combined one so both resolve to a single resident table.
    if not getattr(bacc, "_act_tables_pinned", False):
        orig_get = bacc.get_activation_tables

        def pinned_get(arch):
            t = {k: set(v) for k, v in orig_get(arch).items()}
            exp = mybir.ActivationFunctionType.Exp
            ln = mybir.ActivationFunctionType.Ln
            for name, fns in t.items():
                if name != "natural_log_exp_and_others":
                    fns.discard(exp)
                    fns.discard(ln)
            return t

        bacc.get_activation_tables = pinned_get
        bacc._act_tables_pinned = True

    bf16 = mybir.dt.bfloat16
    f32 = mybir.dt.float32
    Exp = mybir.ActivationFunctionType.Exp
    Ln = mybir.ActivationFunctionType.Ln

    nc = bacc.Bacc()
    xT_d = nc.dram_tensor("xT", [DIM, N], bf16, kind="ExternalInput")
    wqT_d = nc.dram_tensor("wqT", [DIM, FPC], bf16, kind="ExternalInput")
    wkT_d = nc.dram_tensor("wkT", [DIM, FPC], bf16, kind="ExternalInput")
    wvT_d = nc.dram_tensor("wvT", [DIM, FPC], bf16, kind="ExternalInput")
    wpT_d = nc.dram_tensor("wpT", [FPC, DIM], bf16, kind="ExternalInput")
    mask_d = nc.dram_tensor("mask01", [128, 128], bf16, kind="ExternalInput")
    out_d = nc.dram_tensor("out", [N, DIM], bf16, kind="ExternalOutput")

    with tile.TileContext(nc) as tc, ExitStack() as ctx:
        sing = ctx.enter_context(tc.tile_pool(name="sing", bufs=1))
        pspool = ctx.enter_context(tc.tile_pool(name="pspool", bufs=3, space="PSUM"))
        o2pool = ctx.enter_context(tc.tile_pool(name="o2pool", bufs=1, space="PSUM"))
        ptpool = ctx.enter_context(tc.tile_pool(name="ptpool", bufs=3))
        rcpool = ctx.enter_context(tc.tile_pool(name="rcpool", bufs=1))
        outpool = ctx.enter_context(tc.tile_pool(name="outpool", bufs=2))

        xTs = sing.tile([128, KC, N], bf16)
        wqTs = sing.tile([128, KC, FPC], bf16)
        wkTs = sing.tile([128, KC, FPC], bf16)
        wvTs = sing.tile([128, KC, FPC], bf16)
        wpTs = sing.tile([128, 2, DIM], bf16)
        # q/k: [d(128: even head 0:64 / odd 64:128), pair t, 512-col group, 512]
        qTs = sing.tile([128, 2, 4, 512], bf16)
        kTs = sing.tile([128, 2, 4, 512], bf16)
        # v2: per (key block j, head h) a 128-col weight slot:
        # even h -> [V_h | ones], odd h -> [ones | V_h]
        v2 = sing.tile([128, NB, HPC, 128], bf16)
        onorm = sing.tile([128, 2, N], bf16)
        maskS = sing.tile([128, 128], bf16)

        # ---- input DMAs, arrival-ordered to feed the upfront projections:
        # all transfers serialize FIFO on one hw queue at ~266 GB/s, so the
        # order IS the schedule.  x's first 512 cols split in two so the
        # first q-projection starts after ~0.8 MB. ----
        nc.sync.dma_start(out=maskS, in_=mask_d[:, :])
        nc.sync.dma_start(
            out=wqTs[:, :, 0:128],
            in_=wqT_d[:, 0:128].rearrange("(a p) d -> p a d", p=128))
        nc.sync.dma_start(
            out=xTs[:, :, 0:256],
            in_=xT_d[:, 0:256].rearrange("(a p) n -> p a n", p=128))
        nc.sync.dma_start(
            out=wkTs[:, :, 0:128],
            in_=wkT_d[:, 0:128].rearrange("(a p) d -> p a d", p=128))
        nc.sync.dma_start(
            out=xTs[:, :, 256:512],
            in_=xT_d[:, 256:512].rearrange("(a p) n -> p a n", p=128))
        nc.sync.dma_start(
            out=wqTs[:, :, 128:256],
            in_=wqT_d[:, 128:256].rearrange("(a p) d -> p a d", p=128))
        nc.sync.dma_start(
            out=wkTs[:, :, 128:256],
            in_=wkT_d[:, 128:256].rearrange("(a p) d -> p a d", p=128))
        nc.sync.dma_start(
            out=wvTs, in_=wvT_d[:].rearrange("(a p) d -> p a d", p=128))
        for g in range(1, 4):
            n0 = 512 * g
            nc.sync.dma_start(
                out=xTs[:, :, n0:n0 + 512],
                in_=xT_d[:, n0:n0 + 512].rearrange("(a p) n -> p a n", p=128))
        nc.sync.dma_start(out=wpTs, in_=wpT_d[:].rearrange("(a p) d -> p a d", p=128))

        for h in range(HPC):
            ones_cols = slice(64, 128) if h % 2 == 0 else slice(0, 64)
            nc.vector.memset(v2[:, :, h, ones_cols], 1.0)

        # ---- fill units: independent PE work drip-fed into S-run stalls ----
        def qk_unit(wt, dst, t, g, n0=None, w=512):
            """Project w cols (default one 512-col group) of Q or K for pair
            t (8 matmuls)."""
            ps = pspool.tile([128, 512], f32, tag="ps", name="qk_ps")
            if n0 is None:
                n0 = 512 * g
            for k in range(KC):
                nc.tensor.matmul(
                    ps[:, :w],
                    lhsT=wt[:, k, 128 * t:128 * (t + 1)],
                    rhs=xTs[:, k, n0:n0 + w],
                    start=(k == 0), stop=(k == KC - 1),
                )
            nc.vector.tensor_copy(
                out=dst[:, t, g, n0 - 512 * g:n0 - 512 * g + w], in_=ps[:, :w])

        def v_block(j):
            # V_blk[l, d of all 4 heads] = sum_k xT[k-chunk, blk].T @ WvT[k-chunk]
            vps = pspool.tile([128, 256], f32, tag="ps", name="vps")
            for k in range(KC):
                nc.tensor.matmul(
                    vps[:, :],
                    lhsT=xTs[:, k, 128 * j:128 * (j + 1)],
                    rhs=wvTs[:, k, :],
                    start=(k == 0), stop=(k == KC - 1),
                )
            # scatter each pair's two heads into their [V|ones]/[ones|V] slots
            part_d = list(v2[:, :, :, :].ap)[0]
            part_s = list(vps[:, :].ap)[0]
            for t in range(2):
                dst = bass.AP(
                    tensor=v2.tensor,
                    offset=v2.offset + j * HPC * 128 + 256 * t,
                    ap=[[part_d[0], part_d[1]], [192, 2], [1, 64]],
                )
                src = bass.AP(
                    tensor=vps.tensor,
                    offset=vps.offset + 128 * t,
                    ap=[[part_s[0], 128], [64, 2], [1, 64]],
                )
                nc.vector.tensor_copy(out=dst, in_=src)

        def out_proj_nb(nb):
            po = pspool.tile([128, 2, 512], f32, tag="ps", name="po")
            for half in range(2):
                for p in range(2):
                    nc.tensor.matmul(
                        po[:, half, :],
                        lhsT=onorm[:, p, 128 * nb:128 * (nb + 1)],
                        rhs=wpTs[:, p, 512 * half:512 * half + 512],
                        start=(p == 0), stop=(p == 1),
                    )
            ostage = outpool.tile([128, 2, 512], bf16, tag="ostage", name="ostage")
            nc.vector.tensor_copy(out=ostage, in_=po)
            nc.sync.dma_start(
                out=out_d[128 * nb:128 * (nb + 1), :],
                in_=ostage.rearrange("p a b -> p (a b)"),
            )

        # ---- attention pieces ----
        state = {"pt": {}, "o2": {}, "rc": {}}

        def s_block(t, c, j):
            """One key block j of chunk (t, c): S pair matmuls (row-grouped,
            concurrent) + exp + diagonal mask.  Returns its PE deficit."""
            c0 = 512 * c
            o = max(0, 128 * j - c0)
            w = 512 - o
            pt = state["pt"][(t, c)]
            st = pspool.tile([128, 2, 512], f32, tag="ps", name="st")
            for par in range(2):
                nc.tensor.matmul(
                    st[:, par, o:],
                    lhsT=kTs[64 * par:64 * par + 64, t, j // 4,
                             128 * (j % 4):128 * (j % 4) + 128],
                    rhs=qTs[64 * par:64 * par + 64, t, c, o:],
                    start=True, stop=True,
                )
            nc.scalar.activation(
                out=pt[:, j, :, o:], in_=st[:, :, o:],
                func=Exp, scale=SCALE,
            )
            if 128 * j >= c0:  # diagonal block: zero strictly-lower (l>n)
                # on GPSIMD (~410ns/op): keeps the DVE free for casts/norm
                for par in range(2):
                    nc.gpsimd.tensor_mul(
                        pt[:, j, par, o:o + 128],
                        pt[:, j, par, o:o + 128],
                        maskS,
                    )
            # exp time minus the (row-group concurrent) S pair
            return max(0, int((2 * w + 344) / 1.2 + 50 - (w / 2.4 + 190)))

        def o_block(t, c, j):
            """One key block of the O-run for chunk (t, c): 2 matmuls
            accumulating into o2.  The last block triggers norm phase A."""
            c0 = 512 * c
            jc = 4 * c + 4
            o = max(0, 128 * j - c0)
            pt = state["pt"][(t, c)]
            if j == 0:
                state["o2"][(t, c)] = o2pool.tile(
                    [128, 2, 512], f32, tag="o2", name="o2")
            o2 = state["o2"][(t, c)]
            for par in range(2):
                nc.tensor.matmul(
                    o2[:, par, o:],
                    lhsT=v2[:, j, 2 * t + par, :],
                    rhs=pt[:, j, par, o:],
                    start=(j == 0), stop=(j == jc - 1),
                )
            if j == jc - 1:
                norm_a(t, c)

        def norm_a(t, c):
            """Norm phase A: stage row sums to SBUF (DVE; DMA cannot read
            PSUM) and partition-shuffle them onto O's partitions (DMA).
            For (0,3) the O rows are staged too, freeing its o2 PSUM slot
            before the (1,3) self-O-run needs it."""
            o2 = state["o2"][(t, c)]
            rc = rcpool.tile([128, 4, 512], f32, tag="rc", name="rc")
            state["rc"][(t, c)] = rc
            nc.vector.tensor_copy(out=rc[64:128, 0, :], in_=o2[64:128, 0, :])
            nc.vector.tensor_copy(out=rc[0:64, 0, :], in_=o2[0:64, 1, :])
            nc.sync.dma_start(out=rc[0:64, 1, :], in_=rc[64:128, 0, :])
            nc.sync.dma_start(out=rc[64:128, 1, :], in_=rc[0:64, 0, :])
            if (t, c) == (0, 3):
                nc.vector.tensor_copy(out=rc[0:64, 3, :], in_=o2[0:64, 0, :])
                nc.vector.tensor_copy(
                    out=rc[64:128, 3, :], in_=o2[64:128, 1, :])
                state["o2"].pop((t, c))

        def norm_b(t, c, split=1):
            """Norm phase B: 1/s = exp(-ln s) at full 128-partition width on
            ACT, then normalize into onorm on the DVE.  Emitted a couple of
            fill units after phase A so the Ln never head-of-line-blocks the
            scalar queue waiting on the staging copies."""
            c0 = 512 * c
            o2 = state["o2"].pop((t, c), None)
            rc = state["rc"].pop((t, c))
            nc.scalar.activation(out=rc[:, 2, :], in_=rc[:, 1, :], func=Ln)
            nc.scalar.activation(out=rc[:, 1, :], in_=rc[:, 2, :],
                                 func=Exp, scale=-1.0)
            src0 = (lambda s0, s1: o2[0:64, 0, s0:s1]) if o2 is not None \
                else (lambda s0, s1: rc[0:64, 3, s0:s1])
            src1 = (lambda s0, s1: o2[64:128, 1, s0:s1]) if o2 is not None \
                else (lambda s0, s1: rc[64:128, 3, s0:s1])
            for s in range(split):
                w0, w1 = 512 * s // split, 512 * (s + 1) // split
                nc.vector.tensor_mul(
                    out=onorm[0:64, t, c0 + w0:c0 + w1],
                    in0=src0(w0, w1), in1=rc[0:64, 1, w0:w1],
                )
                nc.vector.tensor_mul(
                    out=onorm[64:128, t, c0 + w0:c0 + w1],
                    in0=src1(w0, w1), in1=rc[64:128, 1, w0:w1],
                )

        # ================= static fill schedule =================
        # Chunks run pair-interleaved: (0,0),(1,0),(0,1),(1,1),...  FILLS[i]
        # is the PE work drip-fed into chunk i's S-run (debt-paced, leftovers
        # emitted at S-run end).  O(chunk i) units land in FILLS[i+1]; norm
        # phase B follows two fill units after phase A so its Ln never waits
        # on the staging copies at the head of the scalar queue.
        QK = 1780   # 8x512 matmul unit, ns
        VB = 980    # v_block
        OP = 970    # out_proj_nb

        def qk_u(wt, dst, t, g):
            return (QK, lambda: qk_unit(wt, dst, t, g))

        def v_u(j):
            return (VB, lambda jj=j: v_block(jj))

        def op_u(nb):
            return (OP, lambda b=nb: out_proj_nb(b))

        def nb_u(t, c):
            return (200, lambda: norm_b(t, c))

        def o_us(t, c):
            jc = 4 * c + 4
            return [(int(2 * (512 - max(0, 128 * j - 512 * c)) / 2.4) + 200,
                     (lambda tt=t, cc=c, jj=j: o_block(tt, cc, jj)))
                    for j in range(jc)]

        seq = [(0, 0), (1, 0), (0, 1), (1, 1), (0, 2), (1, 2), (0, 3), (1, 3)]
        FILLS = {
            (0, 0): [qk_u(wqTs, qTs, 1, 0), qk_u(wkTs, kTs, 1, 0),
                     v_u(0), v_u(1), v_u(2), v_u(3)],
            (1, 0): o_us(0, 0)
                    + [qk_u(wqTs, qTs, 0, 1), nb_u(0, 0),
                       qk_u(wkTs, kTs, 0, 1)],
            (0, 1): [qk_u(wqTs, qTs, 1, 1), qk_u(wkTs, kTs, 1, 1)]
                    + o_us(1, 0) + [v_u(4), nb_u(1, 0), v_u(5), v_u(6),
                                    v_u(7)],
            (1, 1): [qk_u(wqTs, qTs, 0, 2), qk_u(wkTs, kTs, 0, 2)]
                    + o_us(0, 1) + [op_u(0), nb_u(0, 1), op_u(1)],
            (0, 2): [qk_u(wqTs, qTs, 1, 2), qk_u(wkTs, kTs, 1, 2)]
                    + o_us(1, 1) + [v_u(8), nb_u(1, 1), v_u(9), v_u(10),
                                    v_u(11), op_u(2), op_u(3)],
            (1, 2): [qk_u(wqTs, qTs, 0, 3), qk_u(wkTs, kTs, 0, 3)]
                    + o_us(0, 2) + [op_u(4), nb_u(0, 2), op_u(5), op_u(6),
                                    op_u(7)],
            (0, 3): [qk_u(wqTs, qTs, 1, 3), qk_u(wkTs, kTs, 1, 3)]
                    + o_us(1, 2) + [v_u(12), nb_u(1, 2), v_u(13), v_u(14),
                                    v_u(15), op_u(8), op_u(9)],
            (1, 3): o_us(0, 3) + [op_u(10), nb_u(0, 3), op_u(11)],
        }

        # upfront: minimal projections for chunk (0,0), in DMA-arrival order
        qk_unit(wqTs, qTs, 0, 0, n0=0, w=256)
        qk_unit(wkTs, kTs, 0, 0, n0=0, w=256)
        qk_unit(wqTs, qTs, 0, 0, n0=256, w=256)
        qk_unit(wkTs, kTs, 0, 0, n0=256, w=256)

        debt = 0
        self_o = 0  # O(1,3) blocks already emitted inside its own S-run
        for t, c in seq:
            fills = list(FILLS[(t, c)])
            state["pt"][(t, c)] = ptpool.tile(
                [128, NB, 2, 512], bf16, tag="pt", name="pt")
            jc = 4 * c + 4
            last = (t, c) == (1, 3)
            for j in range(jc):
                debt += s_block(t, c, j)
                # last chunk: its own O-run trails the S-run once the fill
                # list (which ends past norm_b(0,3)) has drained
                if last and j >= 10 and not fills and self_o < j - 9:
                    o_block(1, 3, self_o)
                    self_o += 1
                    debt -= int(2 * 512 / 2.4) + 200
                while fills and debt >= fills[0][0]:
                    cost, f = fills.pop(0)
                    debt -= cost
                    f()
                debt = min(debt, 4000)
            # leftovers must land before the next chunk's S-run
            for cost, f in fills:
                f()
            # pt of the chunk before last is now fully consumed
            prv = seq[seq.index((t, c)) - 2]
            if seq.index((t, c)) >= 2:
                state["pt"].pop(prv, None)
        # drain: rest of O(1,3) (norm_a fires inside the last block), then
        # the norm with per-nb mults and the final out-projs right behind
        for j in range(self_o, NB):
            o_block(1, 3, j)
        norm_b(1, 3, split=4)
        for nb in (12, 13, 14, 15):
            out_proj_nb(nb)

    nc.finalize()
    return nc


def _get_nc():
    global _built
    if _built is None:
        _built = _build()
    return _built


def make_in_maps(x, Wq, Wk, Wv, Wp):
    # 0 where key>query (strictly-lower in [l, n] coords), else 1
    mask = np.where(
        np.arange(128)[:, None] > np.arange(128)[None, :], 0.0, 1.0
    ).astype(_BF)
    in_maps = []
    for c in range(NCORES):
        b, g = c // HPC, c % HPC
        rows = slice(FPC * g, FPC * (g + 1))
        in_maps.append({
            "xT": np.ascontiguousarray(x[b].T).astype(_BF),
            "wqT": np.ascontiguousarray(Wq[rows, :].T).astype(_BF),
            "wkT": np.ascontiguousarray(Wk[rows, :].T).astype(_BF),
            "wvT": np.ascontiguousarray(Wv[rows, :].T).astype(_BF),
            "wpT": np.ascontiguousarray(Wp[:, rows].T).astype(_BF),
            "mask01": mask,
        })
    return in_maps


def run_sharded(x, Wq, Wk, Wv, Wp, bp, trace=False, **spmd_kwargs):
    from concourse.bass_utils import run_bass_kernel_spmd

    nc = _get_nc()
    in_maps = make_in_maps(x, Wq, Wk, Wv, Wp)
    res = run_bass_kernel_spmd(
        nc, in_maps, core_ids=list(range(NCORES)), trace=trace, **spmd_kwargs
    )
    parts = [r["out"] for r in res.results]
    out = np.zeros((B, N, DIM), np.float32)
    for b in range(B):
        acc = np.zeros((N, DIM), np.float32)
        for g in range(HPC):
            acc += np.asarray(parts[b * HPC + g], dtype=np.float32)
        out[b] = acc + bp.astype(np.float32)[None, :]
    return out, res


def kernel(x, y, Wq, Wk, Wv, Wp, bp):
    x = np.asarray(x, np.float32)
    out, _ = run_sharded(
        x,
        np.asarray(Wq, np.float32), np.asarray(Wk, np.float32),
        np.asarray(Wv, np.float32), np.asarray(Wp, np.float32),
        np.asarray(bp, np.float32),
    )
    return out


# revision 18
# speedup vs baseline: 1.4518x; 1.0078x over previous
"""Causal self-attention block (nn_CrossAttention) on 8 TRN2 NeuronCores.

Sharding: data-parallel over batch (B=2 -> 2 groups of 4 cores), tensor-parallel
over heads within a group (16 heads -> 4 heads/core, splitting Wq/Wk/Wv rows and
Wp columns). Each core computes a full [N, DIM] partial of the output projection
for its 4 heads; the host sums the 4 partials per batch and adds the bias.

Device-side layout ("transposed world", everything feature-major):
  xT   [C=1024, N=2048]     QT/KT = W @ xT -> [d, n] with d on partitions
  V    [l, d] computed DIRECTLY per 128-key-block: V_blk = xT_blk.T @ WvT
       (no PE transposes), then packed per head with a 64-wide ones block
       ([V_h|ones] even heads, [ones|V_h] odd) for fused row-sums.
  S^T  = K_j @ Q^T per (chunk, j) -> [l, n] in PSUM; the two heads of a pair
         run CONCURRENTLY in PE row groups h0/h64 (64-deep contractions).
  P^T  = exp(SCALE*S^T) -> SBUF bf16 (both heads in one ACTIVATE),
         causal-masked by a 0/1 multiply on the diagonal block
  O''  = [V_j|ones].T @ P^T accumulated per 512-query chunk: O rows + row-sums
  1/s  = exp(-ln s), one full-width Ln+Exp pair per chunk (sums staged and
         partition-shuffled onto O's partitions first)
  out  = (O/s).T-pair @ WpT -> [n, e] partial, bf16 to DRAM (host sums in f32)

Schedule: ONE long S-stream over chunks in pair-interleaved order
(0,0),(1,0),(0,1),(1,1),... paced by the scalar engine's exp throughput.
All other PE work -- O-runs of the previous chunk, Q/K/V projections, and
the output projection -- is drip-fed between S matmuls from per-chunk fill
lists sized to the exp-vs-PE deficit, so the PE never idles.  Input DMAs are
split n-major so the first projection starts after ~1.5 MB lands.
No max-subtraction is needed in the softmax (logits*scale max ~8).
"""

import numpy as np
import ml_dtypes

B = 2
N = 2048
DIM = 1024
H = 16
D = 64
SCALE = D ** -0.5
NCORES = 8
HPC = 4          # heads per core
FPC = HPC * D    # feature rows per core (256)

NB = N // 128    # 16 key blocks
KC = DIM // 128  # 8 contraction chunks
NCH = N // 512   # 4 query chunks per pair

_BF = ml_dtypes.bfloat16

_built = None


def _build():
    import concourse.bass as bass
    import concourse.mybir as mybir
    import concourse.tile as tile
    from concourse import bacc
    from contextlib import ExitStack

    # The kernel's only transcendentals are Exp (softmax) and Ln (row-sum
    # reciprocal).  Left to itself the act-table pass picks "exp_and_others"
    # for Exp and "natural_log" for Ln, reloading tables (~1.3us, serializing
    # the scalar engine) on every chunk.  Hide Exp/Ln from every set except
    # the combined one so both resolve to a single resident table.
    if not getattr(bacc, "_act_tables_pinned", False):
        orig_get = bacc.get_activation_tables

        def pinned_get(arch):
            t = {k: set(v) for k, v in orig_get(arch).items()}
            exp = mybir.ActivationFunctionType.Exp
            ln = mybir.ActivationFunctionType.Ln
            for name, fns in t.items():
                if name != "natural_log_exp_and_others":
                    fns.discard(exp)
                    fns.discard(ln)
            return t

        bacc.get_activation_tables = pinned_get
        bacc._act_tables_pinned = True

    bf16 = mybir.dt.bfloat16
    f32 = mybir.dt.float32
    Exp = mybir.ActivationFunctionType.Exp
    Ln = mybir.ActivationFunctionType.Ln

    nc = bacc.Bacc()
    xT_d = nc.dram_tensor("xT", [DIM, N], bf16, kind="ExternalInput")
    wqT_d = nc.dram_tensor("wqT", [DIM, FPC], bf16, kind="ExternalInput")
    wkT_d = nc.dram_tensor("wkT", [DIM, FPC], bf16, kind="ExternalInput")
    wvT_d = nc.dram_tensor("wvT", [DIM, FPC], bf16, kind="ExternalInput")
    wpT_d = nc.dram_tensor("wpT", [FPC, DIM], bf16, kind="ExternalInput")
    mask_d = nc.dram_tensor("mask01", [128, 128], bf16, kind="ExternalInput")
    out_d = nc.dram_tensor("out", [N, DIM], bf16, kind="ExternalOutput")

    with tile.TileContext(nc) as tc, ExitStack() as ctx:
        sing = ctx.enter_context(tc.tile_pool(name="sing", bufs=1))
        pspool = ctx.enter_context(tc.tile_pool(name="pspool", bufs=3, space="PSUM"))
        o2pool = ctx.enter_context(tc.tile_pool(name="o2pool", bufs=1, space="PSUM"))
        ptpool = ctx.enter_context(tc.tile_pool(name="ptpool", bufs=3))
        rcpool = ctx.enter_context(tc.tile_pool(name="rcpool", bufs=1))
        outpool = ctx.enter_context(tc.tile_pool(name="outpool", bufs=2))

        xTs = sing.tile([128, KC, N], bf16)
        wqTs = sing.tile([128, KC, FPC], bf16)
        wkTs = sing.tile([128, KC, FPC], bf16)
        wvTs = sing.tile([128, KC, FPC], bf16)
        wpTs = sing.tile([128, 2, DIM], bf16)
        # q/k: [d(128: even head 0:64 / odd 64:128), pair t, 512-col group, 512]
        qTs = sing.tile([128, 2, 4, 512], bf16)
        kTs = sing.tile([128, 2, 4, 512], bf16)
        # v2: per (key block j, head h) a 128-col weight slot:
        # even h -> [V_h | ones], odd h -> [ones | V_h]
        v2 = sing.tile([128, NB, HPC, 128], bf16)
        onorm = sing.tile([128, 2, N], bf16)
        maskS = sing.tile([128, 128], bf16)

        # ---- input DMAs, arrival-ordered to feed the upfront projections:
        # transfers complete FIFO on the shared ring at ~266 GB/s, so issue
        # order IS the schedule.  wq/wk load full-width (a 128-col slice has
        # 256B elements -> half-rate, so the full 512B-row load costs the
        # same and delivers the t=1 half early).  x's first 512 cols split
        # in two so the first q-projection starts after ~1.2 MB.  x2/x3/wp
        # are issued from inside the chunk loop so mid-stream norm shuffles
        # don't queue behind them. ----
        nc.sync.dma_start(out=maskS, in_=mask_d[:, :])
        nc.sync.dma_start(
            out=wqTs, in_=wqT_d[:].rearrange("(a p) d -> p a d", p=128))
        nc.sync.dma_start(
            out=xTs[:, :, 0:256],
            in_=xT_d[:, 0:256].rearrange("(a p) n -> p a n", p=128))
        nc.sync.dma_start(
            out=wkTs, in_=wkT_d[:].rearrange("(a p) d -> p a d", p=128))
        nc.sync.dma_start(
            out=xTs[:, :, 256:512],
            in_=xT_d[:, 256:512].rearrange("(a p) n -> p a n", p=128))
        nc.sync.dma_start(
            out=wvTs, in_=wvT_d[:].rearrange("(a p) d -> p a d", p=128))
        nc.sync.dma_start(
            out=xTs[:, :, 512:1024],
            in_=xT_d[:, 512:1024].rearrange("(a p) n -> p a n", p=128))

        def dma_x(g):
            n0 = 512 * g
            return lambda: nc.sync.dma_start(
                out=xTs[:, :, n0:n0 + 512],
                in_=xT_d[:, n0:n0 + 512].rearrange("(a p) n -> p a n", p=128))

        def dma_wp():
            nc.sync.dma_start(
                out=wpTs, in_=wpT_d[:].rearrange("(a p) d -> p a d", p=128))

        DEFER_DMA = {1: dma_x(2), 2: dma_x(3), 3: dma_wp}

        for h in range(HPC):
            ones_cols = slice(64, 128) if h % 2 == 0 else slice(0, 64)
            nc.vector.memset(v2[:, :, h, ones_cols], 1.0)

        # ---- fill units: independent PE work drip-fed into S-run stalls ----
        def qk_unit(wt, dst, t, g, n0=None, w=512):
            """Project w cols (default one 512-col group) of Q or K for pair
            t (8 matmuls)."""
            ps = pspool.tile([128, 512], f32, tag="ps", name="qk_ps")
            if n0 is None:
                n0 = 512 * g
            for k in range(KC):
                nc.tensor.matmul(
                    ps[:, :w],
                    lhsT=wt[:, k, 128 * t:128 * (t + 1)],
                    rhs=xTs[:, k, n0:n0 + w],
                    start=(k == 0), stop=(k == KC - 1),
                )
            nc.vector.tensor_copy(
                out=dst[:, t, g, n0 - 512 * g:n0 - 512 * g + w], in_=ps[:, :w])

        def v_block(j):
            # V_blk[l, d of all 4 heads] = sum_k xT[k-chunk, blk].T @ WvT[k-chunk]
            vps = pspool.tile([128, 256], f32, tag="ps", name="vps")
            for k in range(KC):
                nc.tensor.matmul(
                    vps[:, :],
                    lhsT=xTs[:, k, 128 * j:128 * (j + 1)],
                    rhs=wvTs[:, k, :],
                    start=(k == 0), stop=(k == KC - 1),
                )
            # scatter each pair's two heads into their [V|ones]/[ones|V] slots
            part_d = list(v2[:, :, :, :].ap)[0]
            part_s = list(vps[:, :].ap)[0]
            for t in range(2):
                dst = bass.AP(
                    tensor=v2.tensor,
                    offset=v2.offset + j * HPC * 128 + 256 * t,
                    ap=[[part_d[0], part_d[1]], [192, 2], [1, 64]],
                )
                src = bass.AP(
                    tensor=vps.tensor,
                    offset=vps.offset + 128 * t,
                    ap=[[part_s[0], 128], [64, 2], [1, 64]],
                )
                nc.vector.tensor_copy(out=dst, in_=src)

        def out_proj_nb(nb):
            po = pspool.tile([128, 2, 512], f32, tag="ps", name="po")
            for half in range(2):
                for p in range(2):
                    nc.tensor.matmul(
                        po[:, half, :],
                        lhsT=onorm[:, p, 128 * nb:128 * (nb + 1)],
                        rhs=wpTs[:, p, 512 * half:512 * half + 512],
                        start=(p == 0), stop=(p == 1),
                    )
            ostage = outpool.tile([128, 2, 512], bf16, tag="ostage", name="ostage")
            nc.vector.tensor_copy(out=ostage, in_=po)
            nc.sync.dma_start(
                out=out_d[128 * nb:128 * (nb + 1), :],
                in_=ostage.rearrange("p a b -> p (a b)"),
            )

        # ---- attention pieces ----
        state = {"pt": {}, "o2": {}, "rc": {}}

        def s_block(t, c, j):
            """One key block j of chunk (t, c): S pair matmuls (row-grouped,
            concurrent) + exp + diagonal mask.  Returns its PE deficit."""
            c0 = 512 * c
            o = max(0, 128 * j - c0)
            w = 512 - o
            pt = state["pt"][(t, c)]
            st = pspool.tile([128, 2, 512], f32, tag="ps", name="st")
            for par in range(2):
                nc.tensor.matmul(
                    st[:, par, o:],
                    lhsT=kTs[64 * par:64 * par + 64, t, j // 4,
                             128 * (j % 4):128 * (j % 4) + 128],
                    rhs=qTs[64 * par:64 * par + 64, t, c, o:],
                    start=True, stop=True,
                )
            nc.scalar.activation(
                out=pt[:, j, :, o:], in_=st[:, :, o:],
                func=Exp, scale=SCALE,
            )
            if 128 * j >= c0:  # diagonal block: zero strictly-lower (l>n)
                # on GPSIMD (~410ns/op): keeps the DVE free for casts/norm
                for par in range(2):
                    nc.gpsimd.tensor_mul(
                        pt[:, j, par, o:o + 128],
                        pt[:, j, par, o:o + 128],
                        maskS,
                    )
            # exp time minus the (row-group concurrent) S pair
            return max(0, int((2 * w + 344) / 1.2 + 50 - (w / 2.4 + 190)))

        def o_block(t, c, j):
            """One key block of the O-run for chunk (t, c): 2 matmuls
            accumulating into o2.  The last block triggers norm phase A."""
            c0 = 512 * c
            jc = 4 * c + 4
            o = max(0, 128 * j - c0)
            pt = state["pt"][(t, c)]
            if j == 0:
                state["o2"][(t, c)] = o2pool.tile(
                    [128, 2, 512], f32, tag="o2", name="o2")
            o2 = state["o2"][(t, c)]
            for par in range(2):
                nc.tensor.matmul(
                    o2[:, par, o:],
                    lhsT=v2[:, j, 2 * t + par, :],
                    rhs=pt[:, j, par, o:],
                    start=(j == 0), stop=(j == jc - 1),
                )
            if j == jc - 1:
                norm_a(t, c)

        def norm_a(t, c):
            """Norm phase A: stage row sums to SBUF (DVE; DMA cannot read
            PSUM) and partition-shuffle them onto O's partitions (DMA).
            For (0,3) the O rows are staged too, freeing its o2 PSUM slot
            before the (1,3) self-O-run needs it."""
            o2 = state["o2"][(t, c)]
            rc = rcpool.tile([128, 4, 512], f32, tag="rc", name="rc")
            state["rc"][(t, c)] = rc
            nc.vector.tensor_copy(out=rc[64:128, 0, :], in_=o2[64:128, 0, :])
            nc.vector.tensor_copy(out=rc[0:64, 0, :], in_=o2[0:64, 1, :])
            nc.sync.dma_start(out=rc[0:64, 1, :], in_=rc[64:128, 0, :])
            nc.sync.dma_start(out=rc[64:128, 1, :], in_=rc[0:64, 0, :])
            if (t, c) == (0, 3):
                nc.vector.tensor_copy(out=rc[0:64, 3, :], in_=o2[0:64, 0, :])
                nc.vector.tensor_copy(
                    out=rc[64:128, 3, :], in_=o2[64:128, 1, :])
                state["o2"].pop((t, c))
                state["o3free"] = True

        def norm_b(t, c, split=1):
            """Norm phase B: 1/s = exp(-ln s) at full 128-partition width on
            ACT, then normalize into onorm on the DVE.  Emitted a couple of
            fill units after phase A so the Ln never head-of-line-blocks the
            scalar queue waiting on the staging copies."""
            c0 = 512 * c
            o2 = state["o2"].pop((t, c), None)
            rc = state["rc"].pop((t, c))
            nc.scalar.activation(out=rc[:, 2, :], in_=rc[:, 1, :], func=Ln)
            nc.scalar.activation(out=rc[:, 1, :], in_=rc[:, 2, :],
                                 func=Exp, scale=-1.0)
            src0 = (lambda s0, s1: o2[0:64, 0, s0:s1]) if o2 is not None \
                else (lambda s0, s1: rc[0:64, 3, s0:s1])
            src1 = (lambda s0, s1: o2[64:128, 1, s0:s1]) if o2 is not None \
                else (lambda s0, s1: rc[64:128, 3, s0:s1])
            for s in range(split):
                w0, w1 = 512 * s // split, 512 * (s + 1) // split
                nc.vector.tensor_mul(
                    out=onorm[0:64, t, c0 + w0:c0 + w1],
                    in0=src0(w0, w1), in1=rc[0:64, 1, w0:w1],
                )
                nc.vector.tensor_mul(
                    out=onorm[64:128, t, c0 + w0:c0 + w1],
                    in0=src1(w0, w1), in1=rc[64:128, 1, w0:w1],
                )

        # ================= static fill schedule =================
        # Chunks run pair-interleaved: (0,0),(1,0),(0,1),(1,1),...  FILLS[i]
        # is the PE work drip-fed into chunk i's S-run (debt-paced, leftovers
        # emitted at S-run end).  O(chunk i) units land in FILLS[i+1]; norm
        # phase B follows two fill units after phase A so its Ln never waits
        # on the staging copies at the head of the scalar queue.
        QK = 1780   # 8x512 matmul unit, ns
        VB = 980    # v_block
        OP = 970    # out_proj_nb

        def qk_u(wt, dst, t, g):
            return (QK, lambda: qk_unit(wt, dst, t, g))

        def v_u(j):
            return (VB, lambda jj=j: v_block(jj))

        def op_u(nb):
            return (OP, lambda b=nb: out_proj_nb(b))

        def nb_u(t, c):
            return (200, lambda: norm_b(t, c))

        def o_us(t, c):
            jc = 4 * c + 4
            return [(int(2 * (512 - max(0, 128 * j - 512 * c)) / 2.4) + 200,
                     (lambda tt=t, cc=c, jj=j: o_block(tt, cc, jj)))
                    for j in range(jc)]

        seq = [(0, 0), (1, 0), (0, 1), (1, 1), (0, 2), (1, 2), (0, 3), (1, 3)]
        FILLS = {
            (0, 0): [qk_u(wqTs, qTs, 1, 0), qk_u(wkTs, kTs, 1, 0),
                     v_u(0), v_u(1), v_u(2), v_u(3)],
            (1, 0): o_us(0, 0)
                    + [qk_u(wqTs, qTs, 0, 1), nb_u(0, 0),
                       qk_u(wkTs, kTs, 0, 1)],
            (0, 1): [qk_u(wqTs, qTs, 1, 1), qk_u(wkTs, kTs, 1, 1)]
                    + o_us(1, 0) + [v_u(4), nb_u(1, 0), v_u(5), v_u(6),
                                    v_u(7)],
            (1, 1): [qk_u(wqTs, qTs, 0, 2), qk_u(wkTs, kTs, 0, 2)]
                    + o_us(0, 1) + [op_u(0), nb_u(0, 1), op_u(1)],
            (0, 2): [qk_u(wqTs, qTs, 1, 2), qk_u(wkTs, kTs, 1, 2)]
                    + o_us(1, 1) + [v_u(8), nb_u(1, 1), v_u(9), v_u(10),
                                    v_u(11), op_u(2), op_u(3)],
            (1, 2): [qk_u(wqTs, qTs, 0, 3), qk_u(wkTs, kTs, 0, 3)]
                    + o_us(0, 2) + [op_u(4), nb_u(0, 2), op_u(5), op_u(6),
                                    op_u(7)],
            (0, 3): [qk_u(wqTs, qTs, 1, 3), qk_u(wkTs, kTs, 1, 3)]
                    + o_us(1, 2) + [v_u(12), nb_u(1, 2), v_u(13), v_u(14),
                                    v_u(15), op_u(8), op_u(9)],
            # out-projs first so their fat out-DMAs clear the ring before
            # norm(0,3)'s latency-critical shuffles
            (1, 3): [op_u(10), op_u(11)] + o_us(0, 3) + [nb_u(0, 3)],
        }

        # upfront: minimal projections for chunk (0,0), in DMA-arrival order
        qk_unit(wqTs, qTs, 0, 0, n0=0, w=256)
        qk_unit(wkTs, kTs, 0, 0, n0=0, w=256)
        qk_unit(wqTs, qTs, 0, 0, n0=256, w=256)
        qk_unit(wkTs, kTs, 0, 0, n0=256, w=256)

        debt = 0
        self_o = 0  # O(1,3) blocks already emitted inside its own S-run
        for idx, (t, c) in enumerate(seq):
            if idx in DEFER_DMA:
                DEFER_DMA[idx]()
            fills = list(FILLS[(t, c)])
            state["pt"][(t, c)] = ptpool.tile(
                [128, NB, 2, 512], bf16, tag="pt", name="pt")
            jc = 4 * c + 4
            last = (t, c) == (1, 3)
            for j in range(jc):
                debt += s_block(t, c, j)
                while fills and debt >= fills[0][0]:
                    cost, f = fills.pop(0)
                    debt -= cost
                    f()
                # last chunk: its own O-run trails the S-run once norm_a(0,3)
                # has freed the o2 slot (it fires inside o_us(0,3)'s last unit)
                if (last and state.get("o3free") and j >= 9
                        and self_o < j - 8):
                    o_block(1, 3, self_o)
                    self_o += 1
                    debt -= int(2 * 512 / 2.4) + 200
                debt = min(debt, 4000)
            # leftovers must land before the next chunk's S-run
            for cost, f in fills:
                f()
            # pt of the chunk before last is now fully consumed
            prv = seq[seq.index((t, c)) - 2]
            if seq.index((t, c)) >= 2:
                state["pt"].pop(prv, None)
        # drain: rest of O(1,3) (norm_a fires inside the last block), then
        # the norm with per-nb mults and the final out-projs right behind
        for j in range(self_o, NB):
            o_block(1, 3, j)
        norm_b(1, 3, split=4)
        for nb in (12, 13, 14, 15):
            out_proj_nb(nb)

    nc.finalize()
    return nc


def _get_nc():
    global _built
    if _built is None:
        _built = _build()
    return _built


def make_in_maps(x, Wq, Wk, Wv, Wp):
    # 0 where key>query (strictly-lower in [l, n] coords), else 1
    mask = np.where(
        np.arange(128)[:, None] > np.arange(128)[None, :], 0.0, 1.0
    ).astype(_BF)
    in_maps = []
    for c in range(NCORES):
        b, g = c // HPC, c % HPC
        rows = slice(FPC * g, FPC * (g + 1))
        in_maps.append({
            "xT": np.ascontiguousarray(x[b].T).astype(_BF),
            "wqT": np.ascontiguousarray(Wq[rows, :].T).astype(_BF),
            "wkT": np.ascontiguousarray(Wk[rows, :].T).astype(_BF),
            "wvT": np.ascontiguousarray(Wv[rows, :].T).astype(_BF),
            "wpT": np.ascontiguousarray(Wp[:, rows].T).astype(_BF),
            "mask01": mask,
        })
    return in_maps


def run_sharded(x, Wq, Wk, Wv, Wp, bp, trace=False, **spmd_kwargs):
    from concourse.bass_utils import run_bass_kernel_spmd

    nc = _get_nc()
    in_maps = make_in_maps(x, Wq, Wk, Wv, Wp)
    res = run_bass_kernel_spmd(
        nc, in_maps, core_ids=list(range(NCORES)), trace=trace, **spmd_kwargs
    )
    parts = [r["out"] for r in res.results]
    out = np.zeros((B, N, DIM), np.float32)
    for b in range(B):
        acc = np.zeros((N, DIM), np.float32)
        for g in range(HPC):
            acc += np.asarray(parts[b * HPC + g], dtype=np.float32)
        out[b] = acc + bp.astype(np.float32)[None, :]
    return out, res


def kernel(x, y, Wq, Wk, Wv, Wp, bp):
    x = np.asarray(x, np.float32)
    out, _ = run_sharded(
        x,
        np.asarray(Wq, np.float32), np.asarray(Wk, np.float32),
        np.asarray(Wv, np.float32), np.asarray(Wp, np.float32),
        np.asarray(bp, np.float32),
    )
    return out
